# revision 28
# baseline (speedup 1.0000x reference)
"""MoE layer (8 experts, top-2, shared expert) on 8 Trainium2 NeuronCores.

Strategy (expert-parallel, per the sharding hint):
  * Host computes the router (layernorm -> logits -> softmax -> top-2) in
    fp32 numpy — identical math to the reference; this is ~0.1% of the
    layer's FLOPs.  Tokens are gathered per expert on the host ("dispatch")
    and the weighted scatter-add back ("combine") also happens on the host.
  * Core e holds expert e's SwiGLU weights (bf16) and processes the tokens
    routed to expert e (padded to a fixed capacity C_CAP).
  * The always-on shared expert is data-parallel: core c processes tokens
    [1024*c, 1024*(c+1)) with the full shared weights.
  * Device kernel: x @ Wg.T, x @ Wu.T (K=H on partitions), silu*mul fused
    on ACT/DVE, then (gate*up) @ Wd.T, all bf16 inputs with fp32 PSUM
    accumulation.  Tokens ride the matmul free dimension so no transposes
    are needed on device.
  * The aux (load-balance) loss is a scalar computed from the router
    logits — done on host.
"""

import numpy as np
import ml_dtypes

BF16 = ml_dtypes.bfloat16

B, S, H, I, E = 4, 2048, 1024, 2048, 8
T = B * S
P = 128
KH = H // P        # 8  k-tiles over hidden dim
KI = I // P        # 16 k-tiles over intermediate dim
TOP_K = 2
EPS = 1e-5

C_CAP = 2184       # routed-token capacity per expert (= max load; my own
                   # fp32 router determines loads deterministically, and any
                   # overflow falls back to an exact host-side compute)
TS = T // E        # 1024 shared-expert tokens per core
NCHUNK = 512       # token chunk (matmul free dim / one PSUM bank)
WBLK = 1024        # weight columns per packed block
NBLK = I // WBLK   # 2 packed blocks per wg/wu

_PROG = None       # cached compiled Bass program


def _build_program():
    from contextlib import ExitStack

    import concourse.tile as tile
    from concourse import bacc, mybir
    from concourse.bass import ts

    dt = mybir.dt
    nc = bacc.Bacc("TRN2", target_bir_lowering=False, debug=False, num_devices=E)

    xe = nc.dram_tensor("xe", [KH, P, C_CAP], dt.bfloat16, kind="ExternalInput").ap()
    xs = nc.dram_tensor("xs", [KH, P, TS], dt.bfloat16, kind="ExternalInput").ap()
    # Weights arrive pre-packed (host side) so each half loads as ONE DMA
    # with 16 KB contiguous rows: wg/wu block b holds columns
    # [b*WBLK,(b+1)*WBLK) of every k-tile, k-major; wd block b holds
    # k2-tiles 8b..8b+7.
    wg = nc.dram_tensor("wg", [NBLK, P, KH * WBLK], dt.bfloat16,
                        kind="ExternalInput").ap()
    wu = nc.dram_tensor("wu", [NBLK, P, KH * WBLK], dt.bfloat16,
                        kind="ExternalInput").ap()
    wd = nc.dram_tensor("wd", [2, P, 8 * H], dt.bfloat16,
                        kind="ExternalInput").ap()
    wsg = nc.dram_tensor("wsg", [NBLK, P, KH * WBLK], dt.bfloat16,
                         kind="ExternalInput").ap()
    wsu = nc.dram_tensor("wsu", [NBLK, P, KH * WBLK], dt.bfloat16,
                         kind="ExternalInput").ap()
    wsd = nc.dram_tensor("wsd", [2, P, 8 * H], dt.bfloat16,
                         kind="ExternalInput").ap()
    ye = nc.dram_tensor("ye", [KH, P, C_CAP], dt.float32, kind="ExternalOutput").ap()
    ys = nc.dram_tensor("ys", [KH, P, TS], dt.float32, kind="ExternalOutput").ap()

    with tile.TileContext(nc) as tc, ExitStack() as ctx:
        wpool = ctx.enter_context(tc.tile_pool(name="weights", bufs=1))
        xpool = ctx.enter_context(tc.tile_pool(name="xtiles", bufs=2))
        hpool = ctx.enter_context(tc.tile_pool(name="htiles", bufs=2))
        gpool = ctx.enter_context(tc.tile_pool(name="gtiles", bufs=3))
        ypool = ctx.enter_context(tc.tile_pool(name="ytiles", bufs=3))
        pg_pool = ctx.enter_context(tc.tile_pool(name="pg", bufs=2, space="PSUM"))
        pu_pool = ctx.enter_context(tc.tile_pool(name="pu", bufs=2, space="PSUM"))
        py_pool = ctx.enter_context(tc.tile_pool(name="py", bufs=2, space="PSUM"))

        # HAM warm-up: the PE clock sits at 1.2 GHz until ~3.4us of sustained
        # activity.  Burn dummy matmuls during the initial weight-DMA fill so
        # the real matmuls start at 2.4 GHz.
        warm_w = gpool.tile([P, P], dt.bfloat16, tag="warm_w", name="warm_w")
        warm_x = gpool.tile([P, NCHUNK], dt.bfloat16, tag="warm_x", name="warm_x")
        nc.vector.memset(warm_w[:], 0.0)
        nc.vector.memset(warm_x[:], 0.0)
        warm_p = py_pool.tile([P, NCHUNK], dt.float32, tag="warm_p", name="warm_p")
        for _ in range(20):
            nc.tensor.matmul(warm_p[:], warm_w[:], warm_x[:], start=True, stop=True)
        warm_o = gpool.tile([P, NCHUNK], dt.float32, tag="warm_o", name="warm_o")
        nc.vector.tensor_copy(warm_o[:], warm_p[:])

        def swiglu_phase(wg_ap, wu_ap, wd_ap, x_ap, y_ap, ntok):
            # First token chunk before the weight loads (the first gate matmul
            # needs chunk-0 tokens too) and on the gpsimd DGE queues so token
            # and weight streams don't serialize behind each other.
            n0 = min(NCHUNK, ntok)
            x0_sb = xpool.tile([P, KH, NCHUNK], dt.bfloat16, tag="x", name="x_sb")
            for k in range(KH):
                nc.gpsimd.dma_start(out=x0_sb[:, k, :n0], in_=x_ap[k, :, :n0])
            # One DMA per packed weight block (2 MB, 16 KB rows), emitted in
            # first-chunk consumption order: gate b0, up b0, gate b1, up b1,
            # then the down-projection halves.
            wg_sb, wu_sb = [], []
            for b in range(NBLK):
                tg = wpool.tile([P, KH * WBLK], dt.bfloat16, tag=f"wg{b}",
                                name=f"wg_sb{b}")
                nc.sync.dma_start(out=tg[:], in_=wg_ap[b])
                wg_sb.append(tg)
                tu = wpool.tile([P, KH * WBLK], dt.bfloat16, tag=f"wu{b}",
                                name=f"wu_sb{b}")
                nc.sync.dma_start(out=tu[:], in_=wu_ap[b])
                wu_sb.append(tu)
            wd_sb = []
            for b in range(2):
                t = wpool.tile([P, 8 * H], dt.bfloat16, tag=f"wd{b}",
                               name=f"wd_sb{b}")
                nc.sync.dma_start(out=t[:], in_=wd_ap[b])
                wd_sb.append(t)

            MPB = WBLK // P                        # m-tiles per block

            def wg_l(k, m):
                o = k * WBLK + (m % MPB) * P
                return wg_sb[m // MPB][:, o:o + P]

            def wu_l(k, m):
                o = k * WBLK + (m % MPB) * P
                return wu_sb[m // MPB][:, o:o + P]

            def wd_l(k2, m2):
                o = (k2 % 8) * H + m2 * P
                return wd_sb[k2 // 8][:, o:o + P]

            def down_stage(h_sb, c0, n):
                # second matmul of a chunk; emitted one chunk late so PE
                # never waits on the silu/mul tail of the same chunk
                for m2 in range(KH):
                    py = py_pool.tile([P, NCHUNK], dt.float32, tag="py", name="py_t")
                    for k2 in range(KI):
                        nc.tensor.matmul(
                            py[:, :n], wd_l(k2, m2), h_sb[:, k2, :n],
                            start=(k2 == 0), stop=(k2 == KI - 1),
                        )
                    y_sb = ypool.tile([P, NCHUNK], dt.float32, tag="y", name="y_sb")
                    nc.vector.tensor_copy(y_sb[:, :n], py[:, :n])
                    nc.sync.dma_start(out=y_ap[m2, :, c0:c0 + n], in_=y_sb[:, :n])

            pending = None
            for c0 in range(0, ntok, NCHUNK):
                n = min(NCHUNK, ntok - c0)
                if c0 == 0:
                    x_sb = x0_sb
                else:
                    x_sb = xpool.tile([P, KH, NCHUNK], dt.bfloat16, tag="x",
                                      name="x_sb")
                    for k in range(KH):
                        nc.gpsimd.dma_start(out=x_sb[:, k, :n],
                                            in_=x_ap[k, :, c0:c0 + n])
                h_sb = hpool.tile([P, KI, NCHUNK], dt.bfloat16, tag="h", name="h_sb")
                for m in range(KI):
                    pg = pg_pool.tile([P, NCHUNK], dt.float32, tag="pg", name="pg_t")
                    pu = pu_pool.tile([P, NCHUNK], dt.float32, tag="pu", name="pu_t")
                    for k in range(KH):
                        nc.tensor.matmul(
                            pg[:, :n], wg_l(k, m), x_sb[:, k, :n],
                            start=(k == 0), stop=(k == KH - 1),
                        )
                    for k in range(KH):
                        nc.tensor.matmul(
                            pu[:, :n], wu_l(k, m), x_sb[:, k, :n],
                            start=(k == 0), stop=(k == KH - 1),
                        )
                    g_sb = gpool.tile([P, NCHUNK], dt.float32, tag="g", name="g_sb")
                    nc.scalar.activation(
                        g_sb[:, :n], pg[:, :n], mybir.ActivationFunctionType.Silu
                    )
                    nc.vector.tensor_mul(h_sb[:, m, :n], g_sb[:, :n], pu[:, :n])
                if pending is not None:
                    down_stage(*pending)
                pending = (h_sb, c0, n)
            down_stage(*pending)

        # Shared phase first: both phases fill identically, but ending on the
        # routed phase means the kernel tail drains the small 136-token chunk
        # instead of a full 512 one.
        swiglu_phase(wsg, wsu, wsd, xs, ys, TS)
        swiglu_phase(wg, wu, wd, xe, ye, C_CAP)

    nc.compile()
    return nc


def _get_prog():
    global _PROG
    if _PROG is None:
        _PROG = _build_program()
    return _PROG


def _softmax_f32(logits):
    mx = logits.max(-1, keepdims=True)
    ex = np.exp(logits - mx, dtype=np.float32)
    return ex / ex.sum(-1, keepdims=True), mx


def _route(flat, Wr):
    """fp32 numpy replica of the reference router. Returns per-expert token
    index lists, per-expert combine weights, and the raw logits (for aux)."""
    mu = flat.mean(-1, keepdims=True, dtype=np.float32)
    var = np.mean((flat - mu) ** 2, axis=-1, keepdims=True, dtype=np.float32)
    hn = np.clip((flat - mu) / np.sqrt(var + EPS), -100.0, 100.0)
    logits = np.clip(hn @ Wr.T, -20.0, 20.0).astype(np.float32)
    probs, _ = _softmax_f32(logits)
    probs = np.clip(probs, EPS, 1.0)
    order = np.argsort(-probs, axis=-1, kind="stable")
    topi = order[:, :TOP_K]
    topv = np.take_along_axis(probs, topi, axis=1)
    topv = topv / np.clip(topv.sum(-1, keepdims=True), EPS, None)
    idx, wts = [], []
    for e in range(E):
        sel = topi == e
        rows = np.where(sel.any(axis=1))[0]
        w = np.where(sel[rows, 0], topv[rows, 0], topv[rows, 1])
        idx.append(rows)
        wts.append(w.astype(np.float32))
    return idx, wts, logits, topi


def _aux_loss(logits, topi):
    lp, mx = _softmax_f32(logits)
    lp = np.clip(lp, EPS, 1.0)
    counts = np.zeros(E, np.float32)
    for k in range(TOP_K):
        counts += np.bincount(topi[:, k], minlength=E).astype(np.float32)
    tokens_per_expert = counts / float(T * TOP_K)
    avg_probs = lp.mean(0, dtype=np.float32)
    load_balance = E * float((tokens_per_expert * avg_probs).sum())
    lse = (mx[:, 0] + np.log(np.exp(logits - mx, dtype=np.float32)
                             .sum(-1))).astype(np.float32)
    z_loss = float((lse.astype(np.float32) ** 2).mean()) * 0.001
    ps = np.clip(lp, EPS, 1.0 - EPS)
    entropy = float((-(ps * np.log(ps)).sum(-1)).mean())
    entropy_loss = max(np.log(float(E)) - entropy, 0.0) * 0.01
    usage = float((tokens_per_expert > 0.01).mean())
    util_loss = (1.0 - usage) * 0.1
    aux = np.clip(load_balance + z_loss + entropy_loss + util_loss, 0.0, 10.0)
    if np.isnan(aux) or np.isinf(aux):
        aux = 0.1
    return np.float32(aux)


def _host_swiglu(x, Wg_e, Wu_e, Wd_e):
    # fp32 fallback for routed tokens beyond C_CAP (normally never used)
    gate = x @ Wg_e.T
    gate = gate / (1.0 + np.exp(-gate))
    up = x @ Wu_e.T
    return np.clip((gate * up) @ Wd_e.T, -1000.0, 1000.0)


def _pack_gu(w):
    """Wg/Wu [I, H] fp32 -> bf16 packed [NBLK, P, KH*WBLK]:
    block b, partition p, col k*WBLK+c  <-  W.T[k*P+p, b*WBLK+c]."""
    wT = np.ascontiguousarray(w.T.astype(BF16))            # [H, I]
    return np.ascontiguousarray(
        wT.reshape(KH, P, NBLK, WBLK).transpose(2, 1, 0, 3)
    ).reshape(NBLK, P, KH * WBLK)


def _pack_d(w):
    """Wd [H, I] fp32 -> bf16 packed [2, P, 8*H]:
    block b, partition p, col kk*H+c  <-  W.T[(8b+kk)*P+p, c]."""
    wT = np.ascontiguousarray(w.T.astype(BF16))            # [I, H]
    return np.ascontiguousarray(
        wT.reshape(2, 8, P, H).transpose(0, 2, 1, 3)
    ).reshape(2, P, 8 * H)


def _prepare(inputs):
    hs = np.asarray(inputs["hidden_states"], np.float32)
    flat = np.clip(np.nan_to_num(hs.reshape(T, H), nan=0.0,
                                 posinf=1000.0, neginf=-1000.0), -1000.0, 1000.0)
    Wr = np.asarray(inputs["Wr"], np.float32)
    Wg = np.asarray(inputs["Wg"], np.float32)
    Wu = np.asarray(inputs["Wu"], np.float32)
    Wd = np.asarray(inputs["Wd"], np.float32)

    idx, wts, logits, topi = _route(flat, Wr)

    xT = np.ascontiguousarray(flat.astype(BF16).T)       # [H, T] bf16

    wsg_t = _pack_gu(np.asarray(inputs["Wsg"], np.float32))
    wsu_t = _pack_gu(np.asarray(inputs["Wsu"], np.float32))
    wsd_t = _pack_d(np.asarray(inputs["Wsd"], np.float32))

    in_maps = []
    overflow = []
    for c in range(E):
        rows = idx[c]
        use = rows[:C_CAP]
        if len(rows) > C_CAP:
            overflow.append((c, rows[C_CAP:], wts[c][C_CAP:]))
        xe_t = np.zeros((H, C_CAP), BF16)
        xe_t[:, :len(use)] = xT[:, use]
        in_maps.append({
            "xe": xe_t.reshape(KH, P, C_CAP),
            "xs": np.ascontiguousarray(xT[:, c * TS:(c + 1) * TS]).reshape(KH, P, TS),
            "wg": _pack_gu(Wg[c]),
            "wu": _pack_gu(Wu[c]),
            "wd": _pack_d(Wd[c]),
            "wsg": wsg_t,
            "wsu": wsu_t,
            "wsd": wsd_t,
        })

    sig = float(1.0 / (1.0 + np.exp(-np.asarray(inputs["shared_gate"],
                                                np.float32)[0])))
    meta = dict(idx=idx, wts=wts, logits=logits, topi=topi, sig=sig,
                overflow=overflow, flat=flat,
                Wg=Wg, Wu=Wu, Wd=Wd)
    return in_maps, meta


def _combine(results, meta):
    outT = np.zeros((H, T), np.float32)
    for c in range(E):
        rows = meta["idx"][c][:C_CAP]
        ye = results[c]["ye"].reshape(H, C_CAP)
        outT[:, rows] += ye[:, :len(rows)] * meta["wts"][c][None, :len(rows)]
    for c, rows, w in meta["overflow"]:
        yo = _host_swiglu(meta["flat"][rows], meta["Wg"][c], meta["Wu"][c],
                          meta["Wd"][c])
        outT[:, rows] += yo.T * w[None, :]
    for c in range(E):
        ysc = results[c]["ys"].reshape(H, TS)
        outT[:, c * TS:(c + 1) * TS] += meta["sig"] * ysc
    final = np.clip(outT.T, -1000.0, 1000.0).reshape(B, S, H).astype(np.float32)
    aux = _aux_loss(meta["logits"], meta["topi"])
    return final, aux


def _run_device(in_maps, **kwargs):
    from concourse.bass_utils import run_bass_kernel_spmd
    nc = _get_prog()
    return run_bass_kernel_spmd(nc, in_maps, list(range(E)), **kwargs)


def _host_results(meta, inputs):
    """Pure-numpy device-equivalent (fp32) — last-resort fallback."""
    flat = meta["flat"]
    results = []
    for c in range(E):
        rows = meta["idx"][c][:C_CAP]
        ye = np.zeros((C_CAP, H), np.float32)
        ye[:len(rows)] = _host_swiglu(flat[rows], meta["Wg"][c], meta["Wu"][c],
                                      meta["Wd"][c])
        xs = flat[c * TS:(c + 1) * TS]
        ys = _host_swiglu(xs, np.asarray(inputs["Wsg"], np.float32),
                          np.asarray(inputs["Wsu"], np.float32),
                          np.asarray(inputs["Wsd"], np.float32))
        results.append({"ye": np.ascontiguousarray(ye.T).reshape(KH, P, C_CAP),
                        "ys": np.ascontiguousarray(ys.T).reshape(KH, P, TS)})
    return results


def kernel(**inputs):
    in_maps, meta = _prepare(inputs)
    results = None
    for attempt in range(3):
        try:
            results = _run_device(in_maps).results
            break
        except Exception as e:       # transient NRT/axon failures
            import time
            print(f"kernel: device attempt {attempt} failed: {e!r}")
            time.sleep(5)
    if results is None:
        results = _host_results(meta, inputs)
    return _combine(results, meta)


# revision 29
# speedup vs baseline: 1.0192x; 1.0192x over previous
"""MoE layer (8 experts, top-2, shared expert) on 8 Trainium2 NeuronCores.

Strategy (expert-parallel, per the sharding hint):
  * Host computes the router (layernorm -> logits -> softmax -> top-2) in
    fp32 numpy — identical math to the reference; this is ~0.1% of the
    layer's FLOPs.  Tokens are gathered per expert on the host ("dispatch")
    and the weighted scatter-add back ("combine") also happens on the host.
  * Core e holds expert e's SwiGLU weights (bf16) and processes the tokens
    routed to expert e (padded to a fixed capacity C_CAP).
  * The always-on shared expert is data-parallel: core c processes tokens
    [1024*c, 1024*(c+1)) with the full shared weights.
  * Device kernel: x @ Wg.T, x @ Wu.T (K=H on partitions), silu*mul fused
    on ACT/DVE, then (gate*up) @ Wd.T, all bf16 inputs with fp32 PSUM
    accumulation.  Tokens ride the matmul free dimension so no transposes
    are needed on device.
  * The aux (load-balance) loss is a scalar computed from the router
    logits — done on host.
"""

import numpy as np
import ml_dtypes

BF16 = ml_dtypes.bfloat16

B, S, H, I, E = 4, 2048, 1024, 2048, 8
T = B * S
P = 128
KH = H // P        # 8  k-tiles over hidden dim
KI = I // P        # 16 k-tiles over intermediate dim
TOP_K = 2
EPS = 1e-5

C_CAP = 2184       # routed-token capacity per expert (= max load; my own
                   # fp32 router determines loads deterministically, and any
                   # overflow falls back to an exact host-side compute)
TS = T // E        # 1024 shared-expert tokens per core
NCHUNK = 512       # token chunk (matmul free dim / one PSUM bank)

_PROG = None       # cached compiled Bass program


def _build_program():
    from contextlib import ExitStack

    import concourse.tile as tile
    from concourse import bacc, mybir
    from concourse.bass import ts

    dt = mybir.dt
    nc = bacc.Bacc("TRN2", target_bir_lowering=False, debug=False, num_devices=E)

    xe = nc.dram_tensor("xe", [KH, P, C_CAP], dt.bfloat16, kind="ExternalInput").ap()
    xs = nc.dram_tensor("xs", [KH, P, TS], dt.bfloat16, kind="ExternalInput").ap()
    wg = nc.dram_tensor("wg", [KH, P, I], dt.bfloat16, kind="ExternalInput").ap()
    wu = nc.dram_tensor("wu", [KH, P, I], dt.bfloat16, kind="ExternalInput").ap()
    wd = nc.dram_tensor("wd", [KI, P, H], dt.bfloat16, kind="ExternalInput").ap()
    wsg = nc.dram_tensor("wsg", [KH, P, I], dt.bfloat16, kind="ExternalInput").ap()
    wsu = nc.dram_tensor("wsu", [KH, P, I], dt.bfloat16, kind="ExternalInput").ap()
    wsd = nc.dram_tensor("wsd", [KI, P, H], dt.bfloat16, kind="ExternalInput").ap()
    ye = nc.dram_tensor("ye", [KH, P, C_CAP], dt.float32, kind="ExternalOutput").ap()
    ys = nc.dram_tensor("ys", [KH, P, TS], dt.float32, kind="ExternalOutput").ap()

    with tile.TileContext(nc) as tc, ExitStack() as ctx:
        wpool = ctx.enter_context(tc.tile_pool(name="weights", bufs=1))
        xpool = ctx.enter_context(tc.tile_pool(name="xtiles", bufs=2))
        hpool = ctx.enter_context(tc.tile_pool(name="htiles", bufs=2))
        gpool = ctx.enter_context(tc.tile_pool(name="gtiles", bufs=3))
        ypool = ctx.enter_context(tc.tile_pool(name="ytiles", bufs=3))
        pg_pool = ctx.enter_context(tc.tile_pool(name="pg", bufs=2, space="PSUM"))
        pu_pool = ctx.enter_context(tc.tile_pool(name="pu", bufs=2, space="PSUM"))
        py_pool = ctx.enter_context(tc.tile_pool(name="py", bufs=2, space="PSUM"))

        # HAM warm-up: the PE clock sits at 1.2 GHz until ~3.4us of sustained
        # activity.  Burn dummy matmuls during the initial weight-DMA fill so
        # the real matmuls start at 2.4 GHz.
        warm_w = gpool.tile([P, P], dt.bfloat16, tag="warm_w", name="warm_w")
        warm_x = gpool.tile([P, NCHUNK], dt.bfloat16, tag="warm_x", name="warm_x")
        nc.vector.memset(warm_w[:], 0.0)
        nc.vector.memset(warm_x[:], 0.0)
        warm_p = py_pool.tile([P, NCHUNK], dt.float32, tag="warm_p", name="warm_p")
        for _ in range(20):
            nc.tensor.matmul(warm_p[:], warm_w[:], warm_x[:], start=True, stop=True)
        warm_o = gpool.tile([P, NCHUNK], dt.float32, tag="warm_o", name="warm_o")
        nc.vector.tensor_copy(warm_o[:], warm_p[:])

        def swiglu_phase(wg_ap, wu_ap, wd_ap, x_ap, y_ap, ntok):
            # First token chunk before the weight loads (the first gate matmul
            # needs chunk-0 tokens too) and on the gpsimd DGE queues so token
            # and weight streams don't serialize behind each other.
            n0 = min(NCHUNK, ntok)
            x0_sb = xpool.tile([P, KH, NCHUNK], dt.bfloat16, tag="x", name="x_sb")
            for k in range(KH):
                nc.gpsimd.dma_start(out=x0_sb[:, k, :n0], in_=x_ap[k, :, :n0])
            # Stream wg/wu in 1024-column blocks, emitted in the exact order
            # the first chunk's m-loop consumes them (block b covers m-tiles
            # 8b..8b+7, gate before up).  Separate tiles per block give
            # block-granular dependencies, so PE starts after ~3 MB of
            # weights instead of 8 MB.  (256-col blocks degrade DMA to
            # ~200 GB/s — 512 B rows — and are a net loss.)
            WBLK = 1024
            NBLK = I // WBLK                       # 2 column blocks
            wg_sb = [[None] * NBLK for _ in range(KH)]
            wu_sb = [[None] * NBLK for _ in range(KH)]
            for b in range(NBLK):
                for k in range(KH):
                    t = wpool.tile([P, WBLK], dt.bfloat16, tag=f"wg{k}_{b}",
                                   name=f"wg_sb{k}_{b}")
                    nc.sync.dma_start(out=t[:],
                                      in_=wg_ap[k, :, b * WBLK:(b + 1) * WBLK])
                    wg_sb[k][b] = t
                for k in range(KH):
                    t = wpool.tile([P, WBLK], dt.bfloat16, tag=f"wu{k}_{b}",
                                   name=f"wu_sb{k}_{b}")
                    nc.sync.dma_start(out=t[:],
                                      in_=wu_ap[k, :, b * WBLK:(b + 1) * WBLK])
                    wu_sb[k][b] = t
            wd_sb = []
            for k in range(KI):
                t = wpool.tile([P, H], dt.bfloat16, tag=f"wd{k}", name=f"wd_sb{k}")
                nc.sync.dma_start(out=t[:], in_=wd_ap[k])
                wd_sb.append(t)

            MPB = WBLK // P                        # m-tiles per block

            def wg_l(k, m):
                return wg_sb[k][m // MPB][:, (m % MPB) * P:(m % MPB + 1) * P]

            def wu_l(k, m):
                return wu_sb[k][m // MPB][:, (m % MPB) * P:(m % MPB + 1) * P]

            def down_stage(h_sb, c0, n):
                # second matmul of a chunk; emitted one chunk late so PE
                # never waits on the silu/mul tail of the same chunk
                for m2 in range(KH):
                    py = py_pool.tile([P, NCHUNK], dt.float32, tag="py", name="py_t")
                    for k2 in range(KI):
                        nc.tensor.matmul(
                            py[:, :n], wd_sb[k2][:, ts(m2, P)], h_sb[:, k2, :n],
                            start=(k2 == 0), stop=(k2 == KI - 1),
                        )
                    y_sb = ypool.tile([P, NCHUNK], dt.float32, tag="y", name="y_sb")
                    nc.vector.tensor_copy(y_sb[:, :n], py[:, :n])
                    nc.sync.dma_start(out=y_ap[m2, :, c0:c0 + n], in_=y_sb[:, :n])

            pending = None
            for c0 in range(0, ntok, NCHUNK):
                n = min(NCHUNK, ntok - c0)
                if c0 == 0:
                    x_sb = x0_sb
                else:
                    x_sb = xpool.tile([P, KH, NCHUNK], dt.bfloat16, tag="x",
                                      name="x_sb")
                    for k in range(KH):
                        nc.gpsimd.dma_start(out=x_sb[:, k, :n],
                                            in_=x_ap[k, :, c0:c0 + n])
                h_sb = hpool.tile([P, KI, NCHUNK], dt.bfloat16, tag="h", name="h_sb")
                for m in range(KI):
                    pg = pg_pool.tile([P, NCHUNK], dt.float32, tag="pg", name="pg_t")
                    pu = pu_pool.tile([P, NCHUNK], dt.float32, tag="pu", name="pu_t")
                    for k in range(KH):
                        nc.tensor.matmul(
                            pg[:, :n], wg_l(k, m), x_sb[:, k, :n],
                            start=(k == 0), stop=(k == KH - 1),
                        )
                    for k in range(KH):
                        nc.tensor.matmul(
                            pu[:, :n], wu_l(k, m), x_sb[:, k, :n],
                            start=(k == 0), stop=(k == KH - 1),
                        )
                    g_sb = gpool.tile([P, NCHUNK], dt.float32, tag="g", name="g_sb")
                    nc.scalar.activation(
                        g_sb[:, :n], pg[:, :n], mybir.ActivationFunctionType.Silu
                    )
                    nc.vector.tensor_mul(h_sb[:, m, :n], g_sb[:, :n], pu[:, :n])
                if pending is not None:
                    down_stage(*pending)
                pending = (h_sb, c0, n)
            down_stage(*pending)

        # Shared phase first: both phases fill identically, but ending on the
        # routed phase means the kernel tail drains the small 136-token chunk
        # instead of a full 512 one.
        swiglu_phase(wsg, wsu, wsd, xs, ys, TS)
        swiglu_phase(wg, wu, wd, xe, ye, C_CAP)

    nc.compile()
    return nc


def _get_prog():
    global _PROG
    if _PROG is None:
        _PROG = _build_program()
    return _PROG


def _softmax_f32(logits):
    mx = logits.max(-1, keepdims=True)
    ex = np.exp(logits - mx, dtype=np.float32)
    return ex / ex.sum(-1, keepdims=True), mx


def _route(flat, Wr):
    """fp32 numpy replica of the reference router. Returns per-expert token
    index lists, per-expert combine weights, and the raw logits (for aux)."""
    mu = flat.mean(-1, keepdims=True, dtype=np.float32)
    var = np.mean((flat - mu) ** 2, axis=-1, keepdims=True, dtype=np.float32)
    hn = np.clip((flat - mu) / np.sqrt(var + EPS), -100.0, 100.0)
    logits = np.clip(hn @ Wr.T, -20.0, 20.0).astype(np.float32)
    probs, _ = _softmax_f32(logits)
    probs = np.clip(probs, EPS, 1.0)
    order = np.argsort(-probs, axis=-1, kind="stable")
    topi = order[:, :TOP_K]
    topv = np.take_along_axis(probs, topi, axis=1)
    topv = topv / np.clip(topv.sum(-1, keepdims=True), EPS, None)
    idx, wts = [], []
    for e in range(E):
        sel = topi == e
        rows = np.where(sel.any(axis=1))[0]
        w = np.where(sel[rows, 0], topv[rows, 0], topv[rows, 1])
        idx.append(rows)
        wts.append(w.astype(np.float32))
    return idx, wts, logits, topi


def _aux_loss(logits, topi):
    lp, mx = _softmax_f32(logits)
    lp = np.clip(lp, EPS, 1.0)
    counts = np.zeros(E, np.float32)
    for k in range(TOP_K):
        counts += np.bincount(topi[:, k], minlength=E).astype(np.float32)
    tokens_per_expert = counts / float(T * TOP_K)
    avg_probs = lp.mean(0, dtype=np.float32)
    load_balance = E * float((tokens_per_expert * avg_probs).sum())
    lse = (mx[:, 0] + np.log(np.exp(logits - mx, dtype=np.float32)
                             .sum(-1))).astype(np.float32)
    z_loss = float((lse.astype(np.float32) ** 2).mean()) * 0.001
    ps = np.clip(lp, EPS, 1.0 - EPS)
    entropy = float((-(ps * np.log(ps)).sum(-1)).mean())
    entropy_loss = max(np.log(float(E)) - entropy, 0.0) * 0.01
    usage = float((tokens_per_expert > 0.01).mean())
    util_loss = (1.0 - usage) * 0.1
    aux = np.clip(load_balance + z_loss + entropy_loss + util_loss, 0.0, 10.0)
    if np.isnan(aux) or np.isinf(aux):
        aux = 0.1
    return np.float32(aux)


def _host_swiglu(x, Wg_e, Wu_e, Wd_e):
    # fp32 fallback for routed tokens beyond C_CAP (normally never used)
    gate = x @ Wg_e.T
    gate = gate / (1.0 + np.exp(-gate))
    up = x @ Wu_e.T
    return np.clip((gate * up) @ Wd_e.T, -1000.0, 1000.0)


def _wt(w):
    """[R, C] fp32 -> transposed, bf16, partition-tiled [C/P, P, R]."""
    wt = np.ascontiguousarray(w.T.astype(BF16))
    return wt.reshape(w.shape[1] // P, P, w.shape[0])


def _prepare(inputs):
    hs = np.asarray(inputs["hidden_states"], np.float32)
    flat = np.clip(np.nan_to_num(hs.reshape(T, H), nan=0.0,
                                 posinf=1000.0, neginf=-1000.0), -1000.0, 1000.0)
    Wr = np.asarray(inputs["Wr"], np.float32)
    Wg = np.asarray(inputs["Wg"], np.float32)
    Wu = np.asarray(inputs["Wu"], np.float32)
    Wd = np.asarray(inputs["Wd"], np.float32)

    idx, wts, logits, topi = _route(flat, Wr)

    xT = np.ascontiguousarray(flat.astype(BF16).T)       # [H, T] bf16

    wsg_t = _wt(np.asarray(inputs["Wsg"], np.float32))
    wsu_t = _wt(np.asarray(inputs["Wsu"], np.float32))
    wsd_t = _wt(np.asarray(inputs["Wsd"], np.float32))

    in_maps = []
    overflow = []
    for c in range(E):
        rows = idx[c]
        use = rows[:C_CAP]
        if len(rows) > C_CAP:
            overflow.append((c, rows[C_CAP:], wts[c][C_CAP:]))
        xe_t = np.zeros((H, C_CAP), BF16)
        xe_t[:, :len(use)] = xT[:, use]
        in_maps.append({
            "xe": xe_t.reshape(KH, P, C_CAP),
            "xs": np.ascontiguousarray(xT[:, c * TS:(c + 1) * TS]).reshape(KH, P, TS),
            "wg": _wt(Wg[c]),
            "wu": _wt(Wu[c]),
            "wd": _wt(Wd[c]),
            "wsg": wsg_t,
            "wsu": wsu_t,
            "wsd": wsd_t,
        })

    sig = float(1.0 / (1.0 + np.exp(-np.asarray(inputs["shared_gate"],
                                                np.float32)[0])))
    meta = dict(idx=idx, wts=wts, logits=logits, topi=topi, sig=sig,
                overflow=overflow, flat=flat,
                Wg=Wg, Wu=Wu, Wd=Wd)
    return in_maps, meta


def _combine(results, meta):
    outT = np.zeros((H, T), np.float32)
    for c in range(E):
        rows = meta["idx"][c][:C_CAP]
        ye = results[c]["ye"].reshape(H, C_CAP)
        outT[:, rows] += ye[:, :len(rows)] * meta["wts"][c][None, :len(rows)]
    for c, rows, w in meta["overflow"]:
        yo = _host_swiglu(meta["flat"][rows], meta["Wg"][c], meta["Wu"][c],
                          meta["Wd"][c])
        outT[:, rows] += yo.T * w[None, :]
    for c in range(E):
        ysc = results[c]["ys"].reshape(H, TS)
        outT[:, c * TS:(c + 1) * TS] += meta["sig"] * ysc
    final = np.clip(outT.T, -1000.0, 1000.0).reshape(B, S, H).astype(np.float32)
    aux = _aux_loss(meta["logits"], meta["topi"])
    return final, aux


def _run_device(in_maps, **kwargs):
    from concourse.bass_utils import run_bass_kernel_spmd
    nc = _get_prog()
    return run_bass_kernel_spmd(nc, in_maps, list(range(E)), **kwargs)


def _host_results(meta, inputs):
    """Pure-numpy device-equivalent (fp32) — last-resort fallback."""
    flat = meta["flat"]
    results = []
    for c in range(E):
        rows = meta["idx"][c][:C_CAP]
        ye = np.zeros((C_CAP, H), np.float32)
        ye[:len(rows)] = _host_swiglu(flat[rows], meta["Wg"][c], meta["Wu"][c],
                                      meta["Wd"][c])
        xs = flat[c * TS:(c + 1) * TS]
        ys = _host_swiglu(xs, np.asarray(inputs["Wsg"], np.float32),
                          np.asarray(inputs["Wsu"], np.float32),
                          np.asarray(inputs["Wsd"], np.float32))
        results.append({"ye": np.ascontiguousarray(ye.T).reshape(KH, P, C_CAP),
                        "ys": np.ascontiguousarray(ys.T).reshape(KH, P, TS)})
    return results


def kernel(**inputs):
    in_maps, meta = _prepare(inputs)
    results = None
    for attempt in range(3):
        try:
            results = _run_device(in_maps).results
            break
        except Exception as e:       # transient NRT/axon failures
            import time
            print(f"kernel: device attempt {attempt} failed: {e!r}")
            time.sleep(5)
    if results is None:
        results = _host_results(meta, inputs)
    return _combine(results, meta)
